# revision 36
# baseline (speedup 1.0000x reference)
"""Banded multi-head attention on 8 Trainium2 NeuronCores.

Problem: B=2, L=2048, D=1024, H=16 heads, d_k=64. The band mask is a 0/1
FLOAT tensor *added* to the scores (not -inf masked), so the softmax is
dense; exp(s + band) = exp(s) * e^band is handled by multiplying constant
e-or-1 parallelogram tiles over the band region.

Sharding: (batch x 4-head-groups) across the 8 cores. Host pre-transposes
activations/weights (cast to bf16) so every device matmul has its
contraction dim on partitions; the per-core partial output projections are
summed on the host (gather/unshard) together with the output bias.

Schedule: attention (phase D) runs as one flat (head, pass, kb) pipeline,
scalar-engine (exp) paced. The PE's slack inside that loop is filled from a
work queue: the t=1 half of the q/k projections (re-reading resident x
chunks), the back half of the v projection, and the softmax-normalization
chain of the previous pass. Band multiplies run on the otherwise-idle
GPSIMD engine so the vector engine only carries the normalization chain.
"""

import sys

sys.path.insert(0, "/opt/trn_rl_repo")

import numpy as np
import ml_dtypes
from contextlib import ExitStack

import concourse.bass as bass
import concourse.tile as tile
from concourse import bacc, mybir
from concourse.bass_utils import run_bass_kernel_spmd

dt = mybir.dt
AF = mybir.ActivationFunctionType

B, L, D, H, DK = 2, 2048, 1024, 16, 64
HPC = 4            # heads per core
HD = HPC * DK      # 256: head dims per core
NQC, QCW = 4, 512  # q chunks
NKB, KBW = 16, 128 # k blocks
NDC, DCW = 8, 128  # D chunks
SCALE = 1.0 / 8.0  # 1/sqrt(d_k)

_CACHE = {}


def _band_slots(half):
    """delta -> (slot, c0, c1) for 128x512 tiles at k-offset kb*128, q-offset
    qc*512, delta = kb*128 - qc*512. Band cols: f in [delta-half, delta+127+half]."""
    slots = {}
    d = -((half + 127) // 128) * 128
    while d <= half + 511:
        c0, c1 = max(0, d - half), min(512, d + 128 + half)
        if c0 < c1:
            slots[d] = (len(slots), c0, c1)
        d += 128
    return slots


def _build(masksize, stop_after=None):
    half = int(masksize) // 2
    slots = _band_slots(half)
    ns = max(len(slots), 1)

    nc = bacc.Bacc("TRN2", target_bir_lowering=False, debug=False)

    f32, f32r, bf = dt.float32, dt.float32r, dt.bfloat16
    xq = nc.dram_tensor("xq", [D, L], bf, kind="ExternalInput").ap()
    xk = nc.dram_tensor("xk", [D, L], bf, kind="ExternalInput").ap()
    xv = nc.dram_tensor("xv", [D, L], bf, kind="ExternalInput").ap()
    # weights pre-packed on host into SBUF layouts (see _prep_inmaps)
    wq = nc.dram_tensor("wq", [128, NDC * HD], bf, kind="ExternalInput").ap()
    wk = nc.dram_tensor("wk", [128, NDC * HD], bf, kind="ExternalInput").ap()
    wv = nc.dram_tensor("wv", [128, NDC * HD], bf, kind="ExternalInput").ap()
    wo = nc.dram_tensor("wo", [128, 2 * D], bf, kind="ExternalInput").ap()
    bq = nc.dram_tensor("bq", [128, 2], f32, kind="ExternalInput").ap()
    bk = nc.dram_tensor("bk", [128, 2], f32, kind="ExternalInput").ap()
    bv = nc.dram_tensor("bv", [128, HD + 2], bf, kind="ExternalInput").ap()
    em = nc.dram_tensor("em", [128, ns * 512], bf, kind="ExternalInput").ap()
    on1 = nc.dram_tensor("on1", [1, 64], f32, kind="ExternalInput").ap()
    yt = nc.dram_tensor("yt", [D, L], bf, kind="ExternalOutput").ap()

    with tile.TileContext(nc) as tc, ExitStack() as ctx:
        ctx.enter_context(
            nc.allow_low_precision(reason="bf16 matmul path is intentional")
        )
        # ---- persistent SBUF ----
        wts = ctx.enter_context(tc.tile_pool(name="wts", bufs=1))
        big = ctx.enter_context(tc.tile_pool(name="big", bufs=1))
        xr = ctx.enter_context(tc.tile_pool(name="xr", bufs=1))

        wq_sb = wts.tile([128, NDC * HD], bf, tag="wq", name="wq")
        wk_sb = wts.tile([128, NDC * HD], bf, tag="wk", name="wk")
        wv_sb = wts.tile([128, NDC * HD], bf, tag="wv", name="wv")
        wo_sb = wts.tile([128, 2 * D], bf, tag="wo", name="wo")
        bq_sb = wts.tile([128, 2], f32, tag="bq", name="bq")
        bk_sb = wts.tile([128, 2], f32, tag="bk", name="bk")
        bv_sb = wts.tile([128, HD + 2], bf, tag="bv", name="bv")
        em_sb = wts.tile([128, ns * 512], bf, tag="em", name="em")
        on1_sb = wts.tile([1, 64], f32r, tag="on1", name="on1")
        # only what phase B needs up front; everything else is DMA'd later so
        # the x-chunk stream starts immediately
        nc.sync.dma_start(wq_sb[:], wq[:])
        nc.sync.dma_start(wk_sb[:], wk[:])

        # projection outputs (resident): q packed per pair; k per HEAD with
        # the head's 64 dims at their natural partition rows and the other 64
        # rows zeroed — scores then run with a full 128 contraction (a <128
        # contraction halves PE throughput), the zero rows killing the other
        # head's terms in the shared q tile.
        qt_sb = [big.tile([128, L], bf, tag=f"qt{t}", name=f"qt{t}") for t in range(2)]
        kt4_sb = [big.tile([128, L], bf, tag=f"kt{h}", name=f"kt{h}") for h in range(HPC)]
        for h in range(HPC):
            u = h % 2
            nc.vector.memset(kt4_sb[h][(1 - u) * 64:(2 - u) * 64, :], 0.0)
        # attention outputs, packed per head-PAIR on partitions for phase E
        ot_sb = [big.tile([128, L], bf, tag=f"ot{t}", name=f"ot{t}") for t in range(2)]
        # v (natural layout) + ones col per head: [128, HPC*66] per k-block
        vaug_sb = [big.tile([128, HPC * 66], bf, tag=f"vaug{lb}", name=f"vaug{lb}") for lb in range(NKB)]
        for lb in range(NKB):
            for h in range(HPC):
                nc.vector.memset(vaug_sb[lb][:, h * 66 + 64: h * 66 + 66], 1.0)
        # resident x rows, one [128, L] tile per 128-dim chunk (4KB per
        # partition line so the DMA runs near peak; reused by the t=1
        # projection and the v-projection queue items inside phase D)
        xq_r = [xr.tile([128, L], bf, tag=f"xq{c}", name=f"xq{c}") for c in range(NDC)]
        xk_r = [xr.tile([128, L], bf, tag=f"xk{c}", name=f"xk{c}") for c in range(NDC)]
        xv_r = [xr.tile([128, L], bf, tag=f"xv{c}", name=f"xv{c}") for c in range(NDC)]
        for c in range(NDC):
            nc.sync.dma_start(xq_r[c][:], xq[c * DCW:(c + 1) * DCW, :])
            nc.sync.dma_start(xk_r[c][:], xk[c * DCW:(c + 1) * DCW, :])
            if c == 0:
                nc.sync.dma_start(bq_sb[:], bq[:])
                nc.sync.dma_start(bk_sb[:], bk[:])
        nc.sync.dma_start(wv_sb[:], wv[:])
        nc.sync.dma_start(bv_sb[:], bv[:])
        for c in range(NDC):
            nc.sync.dma_start(xv_r[c][:], xv[c * DCW:(c + 1) * DCW, :])
        nc.sync.dma_start(em_sb[:], em[:])
        nc.sync.dma_start(on1_sb[:], on1[:].bitcast(f32r))
        nc.sync.dma_start(wo_sb[:], wo[:])

        # ---- phase B (t=0 half): q/k projections ----
        with tc.tile_pool(name="pqk", bufs=2, space="PSUM") as pqk:
            for qc in range(NQC):
                pq = pqk.tile([128, QCW], f32, tag="pq0", name="pq0")
                pk = pqk.tile([128, QCW], f32, tag="pk0", name="pk0")
                for c in range(NDC):
                    nc.tensor.matmul(
                        pq[:], wq_sb[:, c * HD: c * HD + 128],
                        xq_r[c][:, qc * QCW:(qc + 1) * QCW],
                        start=(c == 0), stop=(c == NDC - 1),
                    )
                    nc.tensor.matmul(
                        pk[:], wk_sb[:, c * HD: c * HD + 128],
                        xk_r[c][:, qc * QCW:(qc + 1) * QCW],
                        start=(c == 0), stop=(c == NDC - 1),
                    )
                nc.vector.tensor_scalar_add(
                    qt_sb[0][:, qc * QCW:(qc + 1) * QCW], pq[:], bq_sb[:, 0:1],
                )
                for u in range(2):
                    nc.vector.tensor_scalar_add(
                        kt4_sb[u][u * 64:(u + 1) * 64, qc * QCW:(qc + 1) * QCW],
                        pk[u * 64:(u + 1) * 64, :], bk_sb[u * 64:(u + 1) * 64, 0:1],
                    )

        # ---- phase C first part (lg 0,1): v projection, c-outer so compute
        # trails the xv DMA stream; one full PSUM bank per (lg, j) group ----
        with tc.tile_pool(name="pvp", bufs=1, space="PSUM") as pvp:
            pv8 = {(lg, j): pvp.tile([128, QCW], f32, tag=f"pv{lg}{j}", name=f"pv{lg}{j}")
                   for lg in range(2) for j in range(4)}
            for c in range(NDC):
                for lg in range(2):
                    for j in range(4):
                        nc.tensor.matmul(
                            pv8[(lg, j)][:, 0:HD],
                            xv_r[c][:, lg * QCW + j * 128: lg * QCW + (j + 1) * 128],
                            wv_sb[:, c * HD:(c + 1) * HD],
                            start=(c == 0), stop=(c == NDC - 1),
                        )
            for lg in range(2):
                for j in range(4):
                    lb = lg * 4 + j
                    for h in range(HPC):
                        nc.vector.tensor_add(
                            vaug_sb[lb][:, h * 66: h * 66 + 64],
                            pv8[(lg, j)][:, h * DK:(h + 1) * DK],
                            bv_sb[:, h * DK:(h + 1) * DK],
                        )

        # ---- phase D: flat (head, pass, kb) pipeline + work queue ----
        with tc.tile_pool(name="psp", bufs=2, space="PSUM") as psp, \
             tc.tile_pool(name="pop", bufs=1, space="PSUM") as pop, \
             tc.tile_pool(name="aux", bufs=1, space="PSUM") as aux, \
             tc.tile_pool(name="ptp", bufs=6) as ptp, \
             tc.tile_pool(name="rcp", bufs=1) as rcp:

            # -- queue items: C second half (lg 2,3), split small so a
            # single slot's PE burst stays ~1us and the scalar engine
            # never starves. One full bank per accumulation group: start=True
            # resets its whole PSUM bank, so two groups never share one. --
            c_state = {}

            def c_sub(lg, jp, cg):
                def run():
                    if cg == 0:
                        c_state[(lg, jp)] = [
                            aux.tile([128, QCW], f32, tag=f"aw{jj}", name=f"cpv{jj}")
                            for jj in range(2)
                        ]
                    pvs = c_state[(lg, jp)]
                    for c in (2 * cg, 2 * cg + 1):
                        for jj in range(2):
                            j = jp * 2 + jj
                            nc.tensor.matmul(
                                pvs[jj][:, 0:HD],
                                xv_r[c][:, lg * QCW + j * 128: lg * QCW + (j + 1) * 128],
                                wv_sb[:, c * HD:(c + 1) * HD],
                                start=(c == 0), stop=(c == NDC - 1),
                            )
                    if cg == 3:
                        for jj in range(2):
                            lb = lg * 4 + jp * 2 + jj
                            for h in range(HPC):
                                nc.vector.tensor_add(
                                    vaug_sb[lb][:, h * 66: h * 66 + 64],
                                    pvs[jj][:, h * DK:(h + 1) * DK],
                                    bv_sb[:, h * DK:(h + 1) * DK],
                                )
                return run

            # -- queue items: B t=1 half (reads resident x rows), split in
            # 4-chunk halves --
            b_state = {}

            def bt1_sub(qc, which, half):
                def run():
                    if half == 0:
                        b_state[(qc, which)] = aux.tile(
                            [128, QCW], f32, tag="aw0", name="bp1"
                        )
                    p1 = b_state[(qc, which)]
                    w_sb, b_sb, src = (
                        (wq_sb, bq_sb, xq_r) if which == "q"
                        else (wk_sb, bk_sb, xk_r)
                    )
                    for c in range(half * 4, half * 4 + 4):
                        nc.tensor.matmul(
                            p1[:], w_sb[:, c * HD + 128: c * HD + 256],
                            src[c][:, qc * QCW:(qc + 1) * QCW],
                            start=(c == 0), stop=(c == NDC - 1),
                        )
                    if half == 1:
                        if which == "q":
                            nc.vector.tensor_scalar_add(
                                qt_sb[1][:, qc * QCW:(qc + 1) * QCW], p1[:], b_sb[:, 1:2],
                            )
                        else:
                            for u in range(2):
                                nc.vector.tensor_scalar_add(
                                    kt4_sb[2 + u][u * 64:(u + 1) * 64, qc * QCW:(qc + 1) * QCW],
                                    p1[u * 64:(u + 1) * 64, :],
                                    b_sb[u * 64:(u + 1) * 64, 1:2],
                                )
                return run

            # -- queue items: softmax normalization for a finished pass --
            def norm_rec_item(ctx_d):
                def run():
                    for i in range(2):
                        sr = rcp.tile([1, QCW], f32, tag=f"sr{i}", name=f"sr{i}")
                        nc.vector.tensor_copy(sr[:], ctx_d["po"][i][64:65, :])
                        rec32 = rcp.tile([1, QCW], f32, tag=f"rc{i}", name=f"rc{i}")
                        nc.vector.reciprocal_approx_fast(rec32[:], sr[:])
                        rec = rcp.tile([1, QCW], f32r, tag=f"re{i}", name=f"re{i}")
                        nc.vector.tensor_copy(rec[:], rec32[:])
                        ctx_d["rec"].append(rec)
                return run

            def norm_fin_item(ctx_d):
                h, p = ctx_d["h"], ctx_d["p"]
                t, po = h // 2, (h % 2) * 64

                def run():
                    for i in range(2):
                        qc = 2 * p + i
                        pbt = aux.tile([128, QCW], f32, tag="aw1", name="pbt")
                        pb = pbt[0:64, :]
                        nc.tensor.matmul(pb, on1_sb[:], ctx_d["rec"][i][:],
                                         start=True, stop=True)
                        bc = rcp.tile([64, QCW], f32, tag=f"bc{i}", name=f"bc{i}")
                        nc.scalar.copy(bc[:], pb)
                        nc.vector.tensor_mul(
                            ot_sb[t][po:po + 64, qc * QCW:(qc + 1) * QCW],
                            ctx_d["po"][i][0:64, :], bc[:],
                        )
                return run

            queue = []
            for lg in (2, 3):
                for jp in range(2):
                    for cg in range(4):
                        queue.append(c_sub(lg, jp, cg))
            for qc in range(NQC):
                for which in ("q", "k"):
                    for half in range(2):
                        queue.append(bt1_sub(qc, which, half))

            items = [(h, p, kb) for h in range(HPC) for p in range(2)
                     for kb in range(NKB)]

            pouts = {}   # (h, p) -> [tile, tile]

            def issue_scores(it):
                h, p, kb = it
                t, po = h // 2, (h % 2) * 64
                ps = psp.tile([128, 2 * QCW], f32, tag="ps", name="ps")
                for i in range(2):
                    qc = 2 * p + i
                    nc.tensor.matmul(
                        ps[:, i * QCW:(i + 1) * QCW],
                        kt4_sb[h][:, kb * KBW:(kb + 1) * KBW],
                        qt_sb[t][:, qc * QCW:(qc + 1) * QCW],
                        start=True, stop=True,
                    )
                pt = ptp.tile([128, 2 * QCW], bf, tag="pt", name="pt")
                nc.scalar.activation(pt[:], ps[:], AF.Exp, scale=SCALE)
                for i in range(2):
                    qc = 2 * p + i
                    delta = kb * KBW - qc * QCW
                    if delta in slots:
                        si, c0, c1 = slots[delta]
                        nc.vector.tensor_mul(
                            pt[:, i * QCW + c0: i * QCW + c1],
                            pt[:, i * QCW + c0: i * QCW + c1],
                            em_sb[:, si * 512 + c0: si * 512 + c1],
                        )
                return pt

            def issue_av(it, pt):
                h, p, kb = it
                if kb == 0:
                    # created here (not in issue_scores) so the previous
                    # pass's copy-out reads are already issued and the WAR
                    # dependency on the shared pout tags is tracked
                    pouts[(h, p)] = [
                        pop.tile([66, QCW], f32, tag=f"pout{i}", name=f"pout{i}")
                        for i in range(2)
                    ]
                for i in range(2):
                    nc.tensor.matmul(
                        pouts[(h, p)][i][:],
                        vaug_sb[kb][:, h * 66:(h + 1) * 66],
                        pt[:, i * QCW:(i + 1) * QCW],
                        start=(kb == 0), stop=(kb == NKB - 1),
                    )

            def pass_done(it):
                """av(kb=15) just issued for pass `it`: copy the psum
                accumulators out (frees the pout tags) and queue the rest of
                the normalization chain."""
                h, p, _ = it
                ctx_d = {"h": h, "p": p, "po": [], "rec": []}
                for i in range(2):
                    po_sb = rcp.tile([66, QCW], f32, tag=f"po{i}", name=f"po{i}")
                    nc.scalar.copy(po_sb[:], pouts[(h, p)][i][:])
                    ctx_d["po"].append(po_sb)
                del pouts[(h, p)]
                # rec first; fin staggered a few slots so its on1-matmul
                # never head-of-line-blocks the PE while the reciprocal
                # chain drains on the vector engine
                queue.insert(min(2, len(queue)), norm_rec_item(ctx_d))
                queue.insert(min(6, len(queue)), norm_fin_item(ctx_d))

            prev_pt = issue_scores(items[0])
            for i in range(1, len(items)):
                for _ in range(2 if i <= 2 else 1):
                    if queue:
                        queue.pop(0)()
                cur_pt = issue_scores(items[i])
                issue_av(items[i - 1], prev_pt)
                if items[i - 1][2] == NKB - 1:
                    pass_done(items[i - 1])
                prev_pt = cur_pt
            issue_av(items[-1], prev_pt)
            pass_done(items[-1])
            for thunk in queue:
                thunk()

        # ---- phase E: output projection yT partial (head pairs packed so
        # the contraction is a full 128) ----
        with tc.tile_pool(name="ysp", bufs=4) as ysp, \
             tc.tile_pool(name="pyp", bufs=4, space="PSUM") as pyp:
            for db in range(NDC):
                for half in range(2):
                    py = pyp.tile([128, 2 * QCW], f32, tag="py", name="py")  # 2 banks
                    for t in range(2):
                        for qq in range(2):
                            qc = half * 2 + qq
                            nc.tensor.matmul(
                                py[:, qq * QCW:(qq + 1) * QCW],
                                wo_sb[:, t * D + db * DCW: t * D + (db + 1) * DCW],
                                ot_sb[t][:, qc * QCW:(qc + 1) * QCW],
                                start=(t == 0), stop=(t == 1),
                            )
                    y_sb = ysp.tile([128, 2 * QCW], bf, tag="y", name="y")
                    nc.vector.tensor_copy(y_sb[:, 0:QCW], py[:, 0:QCW])
                    nc.scalar.copy(y_sb[:, QCW:], py[:, QCW:])
                    nc.sync.dma_start(
                        yt[db * DCW:(db + 1) * DCW, half * 2 * QCW:(half + 1) * 2 * QCW],
                        y_sb[:],
                    )

    nc.compile()
    return nc


def _pack_ndc(w_g):
    """[HD, D] row-slice of a Linear weight -> [128, NDC*HD] SBUF image with
    w[p, c*HD+n] = w_g[n, c*128+p] (lhsT chunks along the free dim)."""
    return np.ascontiguousarray(
        w_g.reshape(HD, NDC, 128).transpose(2, 1, 0).reshape(128, NDC * HD)
    )


def _prep_inmaps(query, key, value, Wq, bq, Wk, bk, Wv, bv, Wo, masksize):
    bf = ml_dtypes.bfloat16
    half = int(masksize) // 2
    slots = _band_slots(half)
    ns = max(len(slots), 1)
    em = np.ones((128, ns * 512), np.float32)
    e1 = np.float32(np.exp(np.float32(1.0)))
    p = np.arange(128)[:, None]
    f = np.arange(512)[None, :]
    for d, (si, _, _) in slots.items():
        em[:, si * 512:(si + 1) * 512] = np.where(
            np.abs(d + p - f) <= half, e1, np.float32(1.0)
        )
    em = em.astype(bf)

    xqT = [np.ascontiguousarray(query[b].T).astype(bf) for b in range(B)]
    xkT = [np.ascontiguousarray(key[b].T).astype(bf) for b in range(B)]
    xvT = [np.ascontiguousarray(value[b].T).astype(bf) for b in range(B)]
    wqP = [_pack_ndc(Wq[g * HD:(g + 1) * HD, :]).astype(bf) for g in range(4)]
    wkP = [_pack_ndc(Wk[g * HD:(g + 1) * HD, :]).astype(bf) for g in range(4)]
    wvP = [_pack_ndc(Wv[g * HD:(g + 1) * HD, :]).astype(bf) for g in range(4)]
    # wo2[p, t*D + db*128 + n] = Wo[db*128+n, g*HD + t*128 + p]
    # (head-PAIR t contraction blocks of 128)
    woP = [
        np.ascontiguousarray(
            Wo[:, g * HD:(g + 1) * HD].reshape(D, 2, 128).transpose(2, 1, 0)
            .reshape(128, 2 * D)
        ).astype(bf)
        for g in range(4)
    ]
    bqP = [np.ascontiguousarray(bq[g * HD:(g + 1) * HD].reshape(2, 128).T) for g in range(4)]
    bkP = [np.ascontiguousarray(bk[g * HD:(g + 1) * HD].reshape(2, 128).T) for g in range(4)]
    bvP = [
        np.ascontiguousarray(
            np.concatenate(
                [np.tile(bv[g * HD:(g + 1) * HD], (128, 1)), np.ones((128, 2), np.float32)],
                axis=1,
            )
        ).astype(bf)
        for g in range(4)
    ]

    in_maps = []
    for c in range(8):
        b, g = c // 4, c % 4
        in_maps.append({
            "xq": xqT[b], "xk": xkT[b], "xv": xvT[b],
            "wq": wqP[g], "wk": wkP[g], "wv": wvP[g], "wo": woP[g],
            "bq": bqP[g], "bk": bkP[g], "bv": bvP[g], "em": em,
            "on1": np.ones((1, 64), np.float32),
        })
    return in_maps


def kernel(query, key, value, Wq, bq, Wk, bk, Wv, bv, Wo, bo, masksize):
    query = np.asarray(query, dtype=np.float32)
    key = np.asarray(key, dtype=np.float32)
    value = np.asarray(value, dtype=np.float32)
    Wq, bq = np.asarray(Wq, np.float32), np.asarray(bq, np.float32)
    Wk, bk = np.asarray(Wk, np.float32), np.asarray(bk, np.float32)
    Wv, bv = np.asarray(Wv, np.float32), np.asarray(bv, np.float32)
    Wo, bo = np.asarray(Wo, np.float32), np.asarray(bo, np.float32)
    ms = int(np.asarray(masksize))

    if ms not in _CACHE:
        _CACHE[ms] = _build(ms)
    nc = _CACHE[ms]

    in_maps = _prep_inmaps(query, key, value, Wq, bq, Wk, bk, Wv, bv, Wo, ms)
    res = run_bass_kernel_spmd(nc, in_maps, list(range(8)))

    out = np.empty((B, L, D), np.float32)
    for b in range(B):
        acc = res.results[4 * b]["yt"].astype(np.float32)
        for g in range(1, 4):
            acc = acc + res.results[4 * b + g]["yt"]
        out[b] = acc.T + bo
    return out


# revision 37
# speedup vs baseline: 1.1426x; 1.1426x over previous
"""Banded multi-head attention on 8 Trainium2 NeuronCores.

Problem: B=2, L=2048, D=1024, H=16 heads, d_k=64. The band mask is a 0/1
FLOAT tensor *added* to the scores (not -inf masked), so the softmax is
dense; exp(s + band) = exp(s) * e^band is handled by multiplying constant
e-or-1 parallelogram tiles over the band region.

Sharding: (batch x 4-head-groups) across the 8 cores. Host pre-transposes
activations/weights (cast to bf16) so every device matmul has its
contraction dim on partitions; the per-core partial output projections are
summed on the host (gather/unshard) together with the output bias.

Schedule: attention (phase D) runs as one flat (head, pass, kb) pipeline,
scalar-engine (exp) paced. The PE's slack inside that loop is filled from a
work queue: the t=1 half of the q/k projections (re-reading resident x
chunks), the back half of the v projection, and the softmax-normalization
chain of the previous pass. Band multiplies run on the otherwise-idle
GPSIMD engine so the vector engine only carries the normalization chain.
"""

import sys

sys.path.insert(0, "/opt/trn_rl_repo")

import numpy as np
import ml_dtypes
from contextlib import ExitStack

import concourse.bass as bass
import concourse.tile as tile
from concourse import bacc, mybir
from concourse.bass_utils import run_bass_kernel_spmd

dt = mybir.dt
AF = mybir.ActivationFunctionType

B, L, D, H, DK = 2, 2048, 1024, 16, 64
HPC = 4            # heads per core
HD = HPC * DK      # 256: head dims per core
NQC, QCW = 4, 512  # q chunks
NKB, KBW = 16, 128 # k blocks
NDC, DCW = 8, 128  # D chunks
SCALE = 1.0 / 8.0  # 1/sqrt(d_k)

_CACHE = {}


def _band_slots(half):
    """delta -> (slot, c0, c1) for 128x512 tiles at k-offset kb*128, q-offset
    qc*512, delta = kb*128 - qc*512. Band cols: f in [delta-half, delta+127+half]."""
    slots = {}
    d = -((half + 127) // 128) * 128
    while d <= half + 511:
        c0, c1 = max(0, d - half), min(512, d + 128 + half)
        if c0 < c1:
            slots[d] = (len(slots), c0, c1)
        d += 128
    return slots


def _build(masksize, stop_after=None):
    half = int(masksize) // 2
    slots = _band_slots(half)
    ns = max(len(slots), 1)

    nc = bacc.Bacc("TRN2", target_bir_lowering=False, debug=False)

    f32, f32r, bf = dt.float32, dt.float32r, dt.bfloat16
    xq = nc.dram_tensor("xq", [D, L], bf, kind="ExternalInput").ap()
    xk = nc.dram_tensor("xk", [D, L], bf, kind="ExternalInput").ap()
    xv = nc.dram_tensor("xv", [D, L], bf, kind="ExternalInput").ap()
    # weights pre-packed on host into SBUF layouts (see _prep_inmaps)
    wq = nc.dram_tensor("wq", [128, NDC * HD], bf, kind="ExternalInput").ap()
    wk = nc.dram_tensor("wk", [128, NDC * HD], bf, kind="ExternalInput").ap()
    wv = nc.dram_tensor("wv", [128, NDC * HD], bf, kind="ExternalInput").ap()
    wo = nc.dram_tensor("wo", [128, 2 * D], bf, kind="ExternalInput").ap()
    bq = nc.dram_tensor("bq", [128, 2], f32, kind="ExternalInput").ap()
    bk = nc.dram_tensor("bk", [128, 2], f32, kind="ExternalInput").ap()
    bv = nc.dram_tensor("bv", [128, HD + 2], bf, kind="ExternalInput").ap()
    em = nc.dram_tensor("em", [128, ns * 512], bf, kind="ExternalInput").ap()
    on1 = nc.dram_tensor("on1", [1, 64], f32, kind="ExternalInput").ap()
    yt = nc.dram_tensor("yt", [D, L], bf, kind="ExternalOutput").ap()

    with tile.TileContext(nc) as tc, ExitStack() as ctx:
        ctx.enter_context(
            nc.allow_low_precision(reason="bf16 matmul path is intentional")
        )
        # ---- persistent SBUF ----
        wts = ctx.enter_context(tc.tile_pool(name="wts", bufs=1))
        big = ctx.enter_context(tc.tile_pool(name="big", bufs=1))
        xr = ctx.enter_context(tc.tile_pool(name="xr", bufs=1))

        wq_sb = wts.tile([128, NDC * HD], bf, tag="wq", name="wq")
        wk_sb = wts.tile([128, NDC * HD], bf, tag="wk", name="wk")
        wv_sb = wts.tile([128, NDC * HD], bf, tag="wv", name="wv")
        wo_sb = wts.tile([128, 2 * D], bf, tag="wo", name="wo")
        bq_sb = wts.tile([128, 2], f32, tag="bq", name="bq")
        bk_sb = wts.tile([128, 2], f32, tag="bk", name="bk")
        bv_sb = wts.tile([128, HD + 2], bf, tag="bv", name="bv")
        em_sb = wts.tile([128, ns * 512], bf, tag="em", name="em")
        on1_sb = wts.tile([1, 64], f32r, tag="on1", name="on1")
        # only what phase B needs up front; everything else is DMA'd later so
        # the x-chunk stream starts immediately
        nc.sync.dma_start(wq_sb[:], wq[:])
        nc.sync.dma_start(wk_sb[:], wk[:])

        # projection outputs (resident): q packed per pair; k per HEAD with
        # the head's 64 dims at their natural partition rows and the other 64
        # rows zeroed — scores then run with a full 128 contraction (a <128
        # contraction halves PE throughput), the zero rows killing the other
        # head's terms in the shared q tile.
        qt_sb = [big.tile([128, L], bf, tag=f"qt{t}", name=f"qt{t}") for t in range(2)]
        kt4_sb = [big.tile([128, L], bf, tag=f"kt{h}", name=f"kt{h}") for h in range(HPC)]
        for h in range(HPC):
            u = h % 2
            nc.vector.memset(kt4_sb[h][(1 - u) * 64:(2 - u) * 64, :], 0.0)
        # attention outputs, packed per head-PAIR on partitions for phase E
        ot_sb = [big.tile([128, L], bf, tag=f"ot{t}", name=f"ot{t}") for t in range(2)]
        # v (natural layout) + ones col per head: [128, HPC*66] per k-block
        vaug_sb = [big.tile([128, HPC * 66], bf, tag=f"vaug{lb}", name=f"vaug{lb}") for lb in range(NKB)]
        for lb in range(NKB):
            for h in range(HPC):
                nc.vector.memset(vaug_sb[lb][:, h * 66 + 64: h * 66 + 66], 1.0)
        # resident x rows, one [128, L] tile per 128-dim chunk (4KB per
        # partition line so the DMA runs near peak; reused by the t=1
        # projection and the v-projection queue items inside phase D)
        xq_r = [xr.tile([128, L], bf, tag=f"xq{c}", name=f"xq{c}") for c in range(NDC)]
        xk_r = [xr.tile([128, L], bf, tag=f"xk{c}", name=f"xk{c}") for c in range(NDC)]
        xv_r = [xr.tile([128, L], bf, tag=f"xv{c}", name=f"xv{c}") for c in range(NDC)]
        for c in range(NDC):
            nc.sync.dma_start(xq_r[c][:], xq[c * DCW:(c + 1) * DCW, :])
            nc.sync.dma_start(xk_r[c][:], xk[c * DCW:(c + 1) * DCW, :])
            if c == 0:
                nc.sync.dma_start(bq_sb[:], bq[:])
                nc.sync.dma_start(bk_sb[:], bk[:])
        nc.sync.dma_start(wv_sb[:], wv[:])
        nc.sync.dma_start(bv_sb[:], bv[:])
        for c in range(NDC):
            nc.sync.dma_start(xv_r[c][:], xv[c * DCW:(c + 1) * DCW, :])
        nc.sync.dma_start(em_sb[:], em[:])
        nc.sync.dma_start(on1_sb[:], on1[:].bitcast(f32r))
        nc.sync.dma_start(wo_sb[:], wo[:])

        # ---- phase B (t=0 half): q/k projections ----
        with tc.tile_pool(name="pqk", bufs=2, space="PSUM") as pqk:
            for qc in range(NQC):
                pq = pqk.tile([128, QCW], f32, tag="pq0", name="pq0")
                pk = pqk.tile([128, QCW], f32, tag="pk0", name="pk0")
                for c in range(NDC):
                    nc.tensor.matmul(
                        pq[:], wq_sb[:, c * HD: c * HD + 128],
                        xq_r[c][:, qc * QCW:(qc + 1) * QCW],
                        start=(c == 0), stop=(c == NDC - 1),
                    )
                    nc.tensor.matmul(
                        pk[:], wk_sb[:, c * HD: c * HD + 128],
                        xk_r[c][:, qc * QCW:(qc + 1) * QCW],
                        start=(c == 0), stop=(c == NDC - 1),
                    )
                nc.vector.tensor_scalar_add(
                    qt_sb[0][:, qc * QCW:(qc + 1) * QCW], pq[:], bq_sb[:, 0:1],
                )
                for u in range(2):
                    nc.vector.tensor_scalar_add(
                        kt4_sb[u][u * 64:(u + 1) * 64, qc * QCW:(qc + 1) * QCW],
                        pk[u * 64:(u + 1) * 64, :], bk_sb[u * 64:(u + 1) * 64, 0:1],
                    )

        # ---- phase C first part (lg 0,1): v projection, c-outer so compute
        # trails the xv DMA stream; one full PSUM bank per (lg, j) group ----
        with tc.tile_pool(name="pvp", bufs=1, space="PSUM") as pvp:
            pv8 = {(lg, j): pvp.tile([128, QCW], f32, tag=f"pv{lg}{j}", name=f"pv{lg}{j}")
                   for lg in range(2) for j in range(4)}
            for c in range(NDC):
                for lg in range(2):
                    for j in range(4):
                        nc.tensor.matmul(
                            pv8[(lg, j)][:, 0:HD],
                            xv_r[c][:, lg * QCW + j * 128: lg * QCW + (j + 1) * 128],
                            wv_sb[:, c * HD:(c + 1) * HD],
                            start=(c == 0), stop=(c == NDC - 1),
                        )
            for lg in range(2):
                for j in range(4):
                    lb = lg * 4 + j
                    for h in range(HPC):
                        nc.vector.tensor_add(
                            vaug_sb[lb][:, h * 66: h * 66 + 64],
                            pv8[(lg, j)][:, h * DK:(h + 1) * DK],
                            bv_sb[:, h * DK:(h + 1) * DK],
                        )

        # ---- phase D: flat (head, pass, kb) pipeline + work queue ----
        with tc.tile_pool(name="psp", bufs=2, space="PSUM") as psp, \
             tc.tile_pool(name="pop", bufs=1, space="PSUM") as pop, \
             tc.tile_pool(name="aux", bufs=1, space="PSUM") as aux, \
             tc.tile_pool(name="ptp", bufs=6) as ptp, \
             tc.tile_pool(name="rcp", bufs=1) as rcp:

            # -- queue items: C second half (lg 2,3), split small so a
            # single slot's PE burst stays ~1us and the scalar engine
            # never starves. One full bank per accumulation group: start=True
            # resets its whole PSUM bank, so two groups never share one. --
            c_state = {}

            def c_sub(lg, jp, cg):
                def run():
                    if cg == 0:
                        c_state[(lg, jp)] = [
                            aux.tile([128, QCW], f32, tag=f"aw{jj}", name=f"cpv{jj}")
                            for jj in range(2)
                        ]
                    pvs = c_state[(lg, jp)]
                    for c in (2 * cg, 2 * cg + 1):
                        for jj in range(2):
                            j = jp * 2 + jj
                            nc.tensor.matmul(
                                pvs[jj][:, 0:HD],
                                xv_r[c][:, lg * QCW + j * 128: lg * QCW + (j + 1) * 128],
                                wv_sb[:, c * HD:(c + 1) * HD],
                                start=(c == 0), stop=(c == NDC - 1),
                            )
                    if cg == 3:
                        for jj in range(2):
                            lb = lg * 4 + jp * 2 + jj
                            for h in range(HPC):
                                nc.vector.tensor_add(
                                    vaug_sb[lb][:, h * 66: h * 66 + 64],
                                    pvs[jj][:, h * DK:(h + 1) * DK],
                                    bv_sb[:, h * DK:(h + 1) * DK],
                                )
                return run

            # -- queue items: B t=1 half (reads resident x rows), split in
            # 4-chunk halves --
            b_state = {}

            def bt1_sub(qc, which, half):
                def run():
                    if half == 0:
                        b_state[(qc, which)] = aux.tile(
                            [128, QCW], f32, tag="aw0", name="bp1"
                        )
                    p1 = b_state[(qc, which)]
                    w_sb, b_sb, src = (
                        (wq_sb, bq_sb, xq_r) if which == "q"
                        else (wk_sb, bk_sb, xk_r)
                    )
                    for c in range(half * 4, half * 4 + 4):
                        nc.tensor.matmul(
                            p1[:], w_sb[:, c * HD + 128: c * HD + 256],
                            src[c][:, qc * QCW:(qc + 1) * QCW],
                            start=(c == 0), stop=(c == NDC - 1),
                        )
                    if half == 1:
                        if which == "q":
                            nc.vector.tensor_scalar_add(
                                qt_sb[1][:, qc * QCW:(qc + 1) * QCW], p1[:], b_sb[:, 1:2],
                            )
                        else:
                            for u in range(2):
                                nc.vector.tensor_scalar_add(
                                    kt4_sb[2 + u][u * 64:(u + 1) * 64, qc * QCW:(qc + 1) * QCW],
                                    p1[u * 64:(u + 1) * 64, :],
                                    b_sb[u * 64:(u + 1) * 64, 1:2],
                                )
                return run

            # -- queue items: softmax normalization for a finished pass --
            def norm_rec_item(ctx_d):
                def run():
                    for i in range(2):
                        sr = rcp.tile([1, QCW], f32, tag=f"sr{i}", name=f"sr{i}")
                        nc.vector.tensor_copy(sr[:], ctx_d["po"][i][64:65, :])
                        rec32 = rcp.tile([1, QCW], f32, tag=f"rc{i}", name=f"rc{i}")
                        nc.vector.reciprocal_approx_fast(rec32[:], sr[:])
                        rec = rcp.tile([1, QCW], f32r, tag=f"re{i}", name=f"re{i}")
                        nc.vector.tensor_copy(rec[:], rec32[:])
                        ctx_d["rec"].append(rec)
                return run

            def norm_fin_item(ctx_d):
                h, p = ctx_d["h"], ctx_d["p"]
                t, po = h // 2, (h % 2) * 64

                def run():
                    for i in range(2):
                        qc = 2 * p + i
                        pbt = aux.tile([128, QCW], f32, tag="aw1", name="pbt")
                        pb = pbt[0:64, :]
                        nc.tensor.matmul(pb, on1_sb[:], ctx_d["rec"][i][:],
                                         start=True, stop=True)
                        bc = rcp.tile([64, QCW], f32, tag=f"bc{i}", name=f"bc{i}")
                        nc.scalar.copy(bc[:], pb)
                        nc.vector.tensor_mul(
                            ot_sb[t][po:po + 64, qc * QCW:(qc + 1) * QCW],
                            ctx_d["po"][i][0:64, :], bc[:],
                        )
                return run

            queue = []
            for lg in (2, 3):
                for jp in range(2):
                    for cg in range(4):
                        queue.append(c_sub(lg, jp, cg))
            for qc in range(NQC):
                for which in ("q", "k"):
                    for half in range(2):
                        queue.append(bt1_sub(qc, which, half))

            items = [(h, p, kb) for h in range(HPC) for p in range(2)
                     for kb in range(NKB)]

            pouts = {}   # (h, p) -> [tile, tile]

            def issue_scores(it):
                h, p, kb = it
                t, po = h // 2, (h % 2) * 64
                ps = psp.tile([128, 2 * QCW], f32, tag="ps", name="ps")
                for i in range(2):
                    qc = 2 * p + i
                    nc.tensor.matmul(
                        ps[:, i * QCW:(i + 1) * QCW],
                        kt4_sb[h][:, kb * KBW:(kb + 1) * KBW],
                        qt_sb[t][:, qc * QCW:(qc + 1) * QCW],
                        start=True, stop=True,
                    )
                pt = ptp.tile([128, 2 * QCW], bf, tag="pt", name="pt")
                nc.scalar.activation(pt[:], ps[:], AF.Exp, scale=SCALE)
                for i in range(2):
                    qc = 2 * p + i
                    delta = kb * KBW - qc * QCW
                    if delta in slots:
                        si, c0, c1 = slots[delta]
                        nc.vector.tensor_mul(
                            pt[:, i * QCW + c0: i * QCW + c1],
                            pt[:, i * QCW + c0: i * QCW + c1],
                            em_sb[:, si * 512 + c0: si * 512 + c1],
                        )
                return pt

            def issue_av(it, pt):
                h, p, kb = it
                if kb == 0:
                    # created here (not in issue_scores) so the previous
                    # pass's copy-out reads are already issued and the WAR
                    # dependency on the shared pout tags is tracked
                    pouts[(h, p)] = [
                        pop.tile([66, QCW], f32, tag=f"pout{i}", name=f"pout{i}")
                        for i in range(2)
                    ]
                for i in range(2):
                    nc.tensor.matmul(
                        pouts[(h, p)][i][:],
                        vaug_sb[kb][:, h * 66:(h + 1) * 66],
                        pt[:, i * QCW:(i + 1) * QCW],
                        start=(kb == 0), stop=(kb == NKB - 1),
                    )

            def pass_done(it):
                """av(kb=15) just issued for pass `it`: copy the psum
                accumulators out (frees the pout tags) and queue the rest of
                the normalization chain."""
                h, p, _ = it
                ctx_d = {"h": h, "p": p, "po": [], "rec": []}
                for i in range(2):
                    po_sb = rcp.tile([66, QCW], f32, tag=f"po{i}", name=f"po{i}")
                    nc.scalar.copy(po_sb[:], pouts[(h, p)][i][:])
                    ctx_d["po"].append(po_sb)
                del pouts[(h, p)]
                # rec first; fin staggered a few slots so its on1-matmul
                # never head-of-line-blocks the PE while the reciprocal
                # chain drains on the vector engine
                queue.insert(min(2, len(queue)), norm_rec_item(ctx_d))
                queue.insert(min(6, len(queue)), norm_fin_item(ctx_d))

            prev_pt = issue_scores(items[0])
            for i in range(1, len(items)):
                for _ in range(2 if i <= 2 else 1):
                    if queue:
                        queue.pop(0)()
                cur_pt = issue_scores(items[i])
                issue_av(items[i - 1], prev_pt)
                if items[i - 1][2] == NKB - 1:
                    pass_done(items[i - 1])
                prev_pt = cur_pt
            issue_av(items[-1], prev_pt)
            pass_done(items[-1])
            for thunk in queue:
                thunk()

        # ---- phase E: output projection yT partial (head pairs packed so
        # the contraction is a full 128) ----
        with tc.tile_pool(name="ysp", bufs=2) as ysp, \
             tc.tile_pool(name="pyp", bufs=2, space="PSUM") as pyp:
            for db in range(NDC):
                py = pyp.tile([128, NQC * QCW], f32, tag="py", name="py")  # 4 banks
                for t in range(2):
                    for qc in range(NQC):
                        nc.tensor.matmul(
                            py[:, qc * QCW:(qc + 1) * QCW],
                            wo_sb[:, t * D + db * DCW: t * D + (db + 1) * DCW],
                            ot_sb[t][:, qc * QCW:(qc + 1) * QCW],
                            start=(t == 0), stop=(t == 1),
                        )
                y_sb = ysp.tile([128, NQC * QCW], bf, tag="y", name="y")
                nc.vector.tensor_copy(y_sb[:, 0:NQC * QCW // 2], py[:, 0:NQC * QCW // 2])
                nc.scalar.copy(y_sb[:, NQC * QCW // 2:], py[:, NQC * QCW // 2:])
                nc.sync.dma_start(yt[db * DCW:(db + 1) * DCW, :], y_sb[:])

    nc.compile()
    return nc


def _pack_ndc(w_g):
    """[HD, D] row-slice of a Linear weight -> [128, NDC*HD] SBUF image with
    w[p, c*HD+n] = w_g[n, c*128+p] (lhsT chunks along the free dim)."""
    return np.ascontiguousarray(
        w_g.reshape(HD, NDC, 128).transpose(2, 1, 0).reshape(128, NDC * HD)
    )


def _prep_inmaps(query, key, value, Wq, bq, Wk, bk, Wv, bv, Wo, masksize):
    bf = ml_dtypes.bfloat16
    half = int(masksize) // 2
    slots = _band_slots(half)
    ns = max(len(slots), 1)
    em = np.ones((128, ns * 512), np.float32)
    e1 = np.float32(np.exp(np.float32(1.0)))
    p = np.arange(128)[:, None]
    f = np.arange(512)[None, :]
    for d, (si, _, _) in slots.items():
        em[:, si * 512:(si + 1) * 512] = np.where(
            np.abs(d + p - f) <= half, e1, np.float32(1.0)
        )
    em = em.astype(bf)

    xqT = [np.ascontiguousarray(query[b].T).astype(bf) for b in range(B)]
    xkT = [np.ascontiguousarray(key[b].T).astype(bf) for b in range(B)]
    xvT = [np.ascontiguousarray(value[b].T).astype(bf) for b in range(B)]
    wqP = [_pack_ndc(Wq[g * HD:(g + 1) * HD, :]).astype(bf) for g in range(4)]
    wkP = [_pack_ndc(Wk[g * HD:(g + 1) * HD, :]).astype(bf) for g in range(4)]
    wvP = [_pack_ndc(Wv[g * HD:(g + 1) * HD, :]).astype(bf) for g in range(4)]
    # wo2[p, t*D + db*128 + n] = Wo[db*128+n, g*HD + t*128 + p]
    # (head-PAIR t contraction blocks of 128)
    woP = [
        np.ascontiguousarray(
            Wo[:, g * HD:(g + 1) * HD].reshape(D, 2, 128).transpose(2, 1, 0)
            .reshape(128, 2 * D)
        ).astype(bf)
        for g in range(4)
    ]
    bqP = [np.ascontiguousarray(bq[g * HD:(g + 1) * HD].reshape(2, 128).T) for g in range(4)]
    bkP = [np.ascontiguousarray(bk[g * HD:(g + 1) * HD].reshape(2, 128).T) for g in range(4)]
    bvP = [
        np.ascontiguousarray(
            np.concatenate(
                [np.tile(bv[g * HD:(g + 1) * HD], (128, 1)), np.ones((128, 2), np.float32)],
                axis=1,
            )
        ).astype(bf)
        for g in range(4)
    ]

    in_maps = []
    for c in range(8):
        b, g = c // 4, c % 4
        in_maps.append({
            "xq": xqT[b], "xk": xkT[b], "xv": xvT[b],
            "wq": wqP[g], "wk": wkP[g], "wv": wvP[g], "wo": woP[g],
            "bq": bqP[g], "bk": bkP[g], "bv": bvP[g], "em": em,
            "on1": np.ones((1, 64), np.float32),
        })
    return in_maps


def kernel(query, key, value, Wq, bq, Wk, bk, Wv, bv, Wo, bo, masksize):
    query = np.asarray(query, dtype=np.float32)
    key = np.asarray(key, dtype=np.float32)
    value = np.asarray(value, dtype=np.float32)
    Wq, bq = np.asarray(Wq, np.float32), np.asarray(bq, np.float32)
    Wk, bk = np.asarray(Wk, np.float32), np.asarray(bk, np.float32)
    Wv, bv = np.asarray(Wv, np.float32), np.asarray(bv, np.float32)
    Wo, bo = np.asarray(Wo, np.float32), np.asarray(bo, np.float32)
    ms = int(np.asarray(masksize))

    if ms not in _CACHE:
        _CACHE[ms] = _build(ms)
    nc = _CACHE[ms]

    in_maps = _prep_inmaps(query, key, value, Wq, bq, Wk, bk, Wv, bv, Wo, ms)
    res = run_bass_kernel_spmd(nc, in_maps, list(range(8)))

    out = np.empty((B, L, D), np.float32)
    for b in range(B):
        acc = res.results[4 * b]["yt"].astype(np.float32)
        for g in range(1, 4):
            acc = acc + res.results[4 * b + g]["yt"]
        out[b] = acc.T + bo
    return out
